# revision 23
# baseline (speedup 1.0000x reference)
"""Trainium2 Bass kernel for the EnrichClassifier pathway MLP.

Network (eval mode, BN folded into weights):
  h1 = relu(x @ (w1*m1).T * s1 + b1')   [8192,5000] -> [8192,4000]
  h2 = relu(h1 @ (w2*m2).T * s2 + b2')                 -> [8192,2000]
  h3 = relu(h2 @ (w3*m3).T * s3 + b3')                 -> [8192,1000]
  sc = relu(h3 @ (w4*m4).T + b4)                       -> [8192,200]
  out = sc @ wc.T + bc                                 -> [8192,50]

Structure: m1 gives each of 200 pathways a private set of 100 genes;
20 L1 units per pathway share that set. m2/m3/m4 are block-diagonal
(20->10->5->1 per pathway). Effective work is ~7.5 GFLOP instead of
the dense 495 GFLOP.

Key implementation choices:
- x is pre-gathered on the host into a partition-major fp8-e3m4 tensor
  xg[NT, 128, NPATH, NB]: slot (nt, k, p, b) = x^T[gene_k(p), 512*nt+b].
  The kernel streams xg with plain sequential HWDGE DMA in multi-
  supergroup chunks (no dma_gather; full-line-rate contiguous reads).
- All biases ride constant-1 lanes: gene slot 127 of every pathway is
  1.0 and row 127 of each stationary matrix carries the folded bias
  (and a passthrough 1 so deeper layers see a constant lane too).
  Every activation is then a pure relu/copy, so PSUM tiles can be
  relu'd in batched [128, 2*512] instructions, split between the
  Scalar and Vector engines.
- L1 runs as per-pathway [128x32] matmuls packed 4-wide into the PE
  array via column tiling; L2/L3/L4/classifier are block-packed
  [128x128] matmuls accumulating over input tiles.

Sharding: pure data parallel over batch across the 8 cores (1024 rows
per core); packed weights replicated.
"""

import contextlib

import numpy as np
import ml_dtypes

import concourse.bass as bass
import concourse.bacc as bacc
import concourse.tile as tile
import concourse.mybir as mybir
from concourse.bass_utils import run_bass_kernel_spmd

# ---------------- hardcoded geometry ----------------
B, G, NPATH = 8192, 5000, 200
NCORES = 8
BC = B // NCORES            # 1024 rows per core
NT = 2                      # batch tiles per core
NB = BC // NT               # 512 = PSUM bank free size (fp32)
U1, U2, U3 = 20, 10, 5      # per-pathway units per layer
NL = 50                     # labels
KG = 104                    # gene slots/pathway: 100 genes, const-1 at 100,
                            # zero pad 101-103 (DMA partition count % 8 == 0)
SGS = 12                    # pathways per supergroup
NSG = 17                    # supergroups (16 full + 1 of 8)
NQUAD = 50                  # h1 tiles (4 pathways each)
NPAIR = 9                   # h3 tiles (24 pathways each, last 8)
CHUNKS = [(0, 6), (6, 6), (12, 5)]   # gather chunks: (first sg, n sgs)
F32 = mybir.dt.float32
F32R = mybir.dt.float32r
F16 = mybir.dt.float16
F8 = mybir.dt.float8e3      # e3m4
F8NP = ml_dtypes.float8_e3m4
RELU = mybir.ActivationFunctionType.Relu
MAX = mybir.AluOpType.max

_COMPILED = None  # cached across calls


def _sg_paths(sg):
    return range(SGS * sg, min(SGS * sg + SGS, NPATH))


def _sg_quads(sg):
    return 3 if sg < NSG - 1 else 2


def _quad_sg(t):
    return (t // 3, t % 3) if t < 48 else (16, t - 48)


def _pack(inputs):
    """Host-side packing: BN folding, per-pathway weight blocks with bias
    lanes, per-core pre-gathered x slices. Pure layout/folding/casting."""
    f = lambda k: np.asarray(inputs[k], np.float32)
    x = f("x")
    w1, b1, m1 = f("w1"), f("b1"), f("m1")
    w2, b2, m2 = f("w2"), f("b2"), f("m2")
    w3, b3, m3 = f("w3"), f("b3"), f("m3")
    w4, b4, m4 = f("w4"), f("b4"), f("m4")
    wc, bc = f("wc"), f("bc")

    def fold(gamma, beta, rm, rv):
        s = gamma / np.sqrt(rv + 1e-5)
        return s, beta - rm * s

    s1, t1 = fold(f("gamma1"), f("beta1"), f("rm1"), f("rv1"))
    s2, t2 = fold(f("gamma2"), f("beta2"), f("rm2"), f("rv2"))
    s3, t3 = fold(f("gamma3"), f("beta3"), f("rm3"), f("rv3"))
    w1m = w1 * m1 * s1[:, None]
    b1f = b1 * s1 + t1
    w2m = w2 * m2 * s2[:, None]
    b2f = b2 * s2 + t2
    w3m = w3 * m3 * s3[:, None]
    b3f = b3 * s3 + t3
    w4m = w4 * m4

    # per-pathway gene index table (each pathway has exactly 100 genes);
    # slot 100 is the constant-1 lane (row G of the augmented x), slots
    # 101..103 are zero padding (row G+1)
    genes = []
    idx_pad = np.full(NPATH * KG, G + 1, np.int64)
    for p in range(NPATH):
        g = np.nonzero(m1[U1 * p] != 0)[0]
        assert len(g) == 100
        genes.append(g)
        idx_pad[KG * p : KG * p + len(g)] = g
        idx_pad[KG * p + 100] = G

    # L1 stationary [KG, 32*NPATH]: col 32p+u = unit u of pathway p,
    # row k = k-th gathered gene of pathway p; row 100 = bias lane.
    w1s = np.zeros((KG, 32 * NPATH), np.float16)
    for p in range(NPATH):
        g = genes[p]
        w1s[: len(g), 32 * p : 32 * p + U1] = w1m[U1 * p : U1 * p + U1, g].T.astype(np.float16)
        w1s[100, 32 * p : 32 * p + U1] = b1f[U1 * p : U1 * p + U1]
    for t in range(NQUAD):
        w1s[100, 32 * (4 * t + 3) + 31] = 1.0   # h1 const lane (partition 127)

    # L2 stationary per h1 tile t (pathways 4t..4t+3): [128,128]
    # rows 32j+u = h1 unit u of pathway 4t+j ; cols 10l+v, l = sg-local path
    w2s = np.zeros((128, 128 * NQUAD), np.float32)
    for t in range(NQUAD):
        for j in range(4):
            p = 4 * t + j
            l = p - SGS * (p // SGS)
            blk = w2m[U2 * p : U2 * p + U2, U1 * p : U1 * p + U1]  # [10,20]
            w2s[32 * j : 32 * j + U1, 128 * t + U2 * l : 128 * t + U2 * l + U2] = blk.T
    for sg in range(NSG):
        base = 128 * (3 * sg if sg < 16 else 48)  # the sg's g==0 quad tile
        for l, p in enumerate(_sg_paths(sg)):
            w2s[127, base + U2 * l : base + U2 * l + U2] = b2f[U2 * p : U2 * p + U2]
        w2s[127, base + 127] = 1.0

    # L3 stationary per h2 tile sg: rows 10l+v, cols 5q+w (q = pair-local)
    w3s = np.zeros((128, 128 * NSG), np.float32)
    for sg in range(NSG):
        for l, p in enumerate(_sg_paths(sg)):
            q = SGS * (sg % 2) + l
            blk = w3m[U3 * p : U3 * p + U3, U2 * p : U2 * p + U2]  # [5,10]
            w3s[U2 * l : U2 * l + U2, 128 * sg + U3 * q : 128 * sg + U3 * q + U3] = blk.T
    for pr in range(NPAIR):
        base = 128 * (2 * pr)  # even sg of the pair
        for p in range(24 * pr, min(24 * pr + 24, NPATH)):
            q = p - 24 * pr
            w3s[127, base + U3 * q : base + U3 * q + U3] = b3f[U3 * p : U3 * p + U3]
        w3s[127, base + 127] = 1.0

    # L4 stationary per h3 tile i: rows 5q+w, col 24*(i%5)+q (A: i<5, B: i>=5)
    w4s = np.zeros((128, 128 * NPAIR), np.float32)
    for i in range(NPAIR):
        base = 24 * i if i < 5 else 24 * (i - 5)
        for p in range(24 * i, min(24 * i + 24, NPATH)):
            q = p - 24 * i
            w4s[U3 * q : U3 * q + U3, 128 * i + base + q] = w4m[p, U3 * p : U3 * p + U3]
    w4s[127, 0:120] = b4[:120]
    w4s[127, 127] = 1.0
    w4s[127, 128 * 5 : 128 * 5 + 80] = b4[120:]
    w4s[127, 128 * 5 + 127] = 1.0

    # classifier stationary per scores tile T: rows r = pathway 120T+r;
    # row 127 of the T=0 block carries bc.
    wcs = np.zeros((128, 2 * 64), np.float32)
    wcs[:120, :NL] = wc[:, :120].T
    wcs[:80, 64 : 64 + NL] = wc[:, 120:].T
    wcs[127, :NL] = bc

    shared = {
        "w1s": w1s, "w2s": w2s, "w3s": w3s, "w4s": w4s, "wcs": wcs,
    }
    in_maps = []
    for c in range(NCORES):
        m = dict(shared)
        xc = x[BC * c : BC * (c + 1)].T  # [5000, 1024]
        xq = np.concatenate(
            [xc.astype(F8NP), np.ones((1, BC), F8NP),
             np.zeros((1, BC), F8NP)], axis=0)  # row G = 1, row G+1 = 0
        # [NPATH*KG, 1024] -> [NPATH, KG, NT, NB] -> [NT, KG, NPATH, NB]
        m["xg"] = np.ascontiguousarray(
            xq[idx_pad].reshape(NPATH, KG, NT, NB).transpose(2, 1, 0, 3))
        in_maps.append(m)
    return in_maps


def _build(repeat=None, mode="full"):
    """Build + compile the per-core Bass program (shared across cores).

    repeat: if set, wrap the whole compute body in an on-device For_i loop
    (used only for timing measurements; outputs are identical).
    mode: "full" | "dma" (gather DMAs only) | "compute" (no gather DMAs;
    L1 reads a single preloaded chunk) — perf probes only."""
    nc = bacc.Bacc("TRN2", target_bir_lowering=False, debug=False,
                   enable_asserts=False)

    dram_in = {}
    for name, shape, dt_ in [
        ("xg", [NT, KG, NPATH, NB], F8), ("w1s", [KG, 32 * NPATH], F16),
        ("w2s", [128, 128 * NQUAD], F32R), ("w3s", [128, 128 * NSG], F32R),
        ("w4s", [128, 128 * NPAIR], F32R), ("wcs", [128, 2 * 64], F32R),
    ]:
        dram_in[name] = nc.dram_tensor(name, shape, dt_, kind="ExternalInput").ap()
    # out is stored transposed [NL, BC]; the host transposes it back
    out_d = nc.dram_tensor("out", [NL, BC], F32, kind="ExternalOutput").ap()

    with tile.TileContext(nc) as tc:
        const = tc.alloc_tile_pool(name="const", bufs=1, space="SBUF")
        cs = {}
        for name, ap in dram_in.items():
            if name == "xg":
                continue  # streamed from DRAM per chunk
            t = const.tile(ap.shape, ap.dtype, name=f"c_{name}")
            nc.sync.dma_start(t[:], ap[:])
            cs[name] = t

        gpool = tc.alloc_tile_pool(name="gath", bufs=2, space="SBUF")
        if mode == "compute":
            gfix = const.tile([KG, 72, NB], F8, name="gfix")
            nc.sync.dma_start(gfix[:], dram_in["xg"][0][:, 0:72, :])
        h1p = tc.alloc_tile_pool(name="h1", bufs=3, space="SBUF")
        h2p = tc.alloc_tile_pool(name="h2", bufs=3, space="SBUF")
        h3p = tc.alloc_tile_pool(name="h3", bufs=3, space="SBUF")
        scp = tc.alloc_tile_pool(name="sc", bufs=3, space="SBUF")
        otp = tc.alloc_tile_pool(name="ot", bufs=2, space="SBUF")
        ps1 = tc.alloc_tile_pool(name="ps1", bufs=2, space="PSUM")  # 2 banks/tile
        ps2 = tc.alloc_tile_pool(name="ps2", bufs=2, space="PSUM")
        ps3 = tc.alloc_tile_pool(name="ps3", bufs=1, space="PSUM")
        ps4 = tc.alloc_tile_pool(name="ps4", bufs=1, space="PSUM")  # also pc/pt

        loop = tc.For_i(0, repeat, 1) if repeat else contextlib.nullcontext()
        with loop:
            for nt in range(NT):
                st = dict(p2=None, h2_pair=[], sc_tiles=[], p4=None,
                          nrelu=0, n2=0, n3=0)

                def relu(out_ap, in_ap, on_act):
                    if on_act:
                        nc.scalar.activation(out_ap, in_ap, RELU)
                    else:
                        nc.vector.tensor_scalar_max(out_ap, in_ap, 0.0)

                def finish_sg(sg):
                    h2 = h2p.tile([128, NB], F32R, name="h2t", tag="h2t")
                    relu(h2[:], st["p2"][:], st["n2"] % 2 == 0)
                    st["n2"] += 1
                    st["h2_pair"].append((sg, h2))
                    if sg % 2 == 1 or sg == NSG - 1:
                        pr = sg // 2
                        p3 = ps3.tile([128, NB], F32, name="p3", tag="p3")
                        for k, (sgi, h2t) in enumerate(st["h2_pair"]):
                            nc.tensor.matmul(
                                p3[:], (cs["w3s"][:, 128 * sgi : 128 * (sgi + 1)]),
                                h2t[:], start=(k == 0),
                                stop=(k == len(st["h2_pair"]) - 1),
                            )
                        st["h2_pair"] = []
                        h3 = h3p.tile([128, NB], F32R, name="h3t", tag="h3t")
                        relu(h3[:], p3[:], st["n3"] % 2 == 0)
                        st["n3"] += 1
                        # ---- L4: scores tile A (h3 tiles 0-4), B (5-8) ----
                        grp_end = (pr == 4) or (pr == NPAIR - 1)
                        T = 0 if pr < 5 else 1
                        first = pr == 0 or pr == 5
                        if first:
                            st["p4"] = ps4.tile([128, NB], F32, name="p4", tag="p4")
                        nc.tensor.matmul(
                            st["p4"][:], (cs["w4s"][:, 128 * pr : 128 * (pr + 1)]),
                            h3[:], start=first, stop=grp_end,
                        )
                        if grp_end:
                            sc = scp.tile([128, NB], F32R, name="sct", tag="sct")
                            relu(sc[:], st["p4"][:], True)
                            st["sc_tiles"].append((T, sc))

                def emit_l2(t, h1_ap):
                    sg, g = _quad_sg(t)
                    nq = _sg_quads(sg)
                    if g == 0:
                        st["p2"] = ps2.tile([128, NB], F32, name="p2", tag="p2")
                    nc.tensor.matmul(
                        st["p2"][:], (cs["w2s"][:, 128 * t : 128 * (t + 1)]),
                        h1_ap, start=(g == 0), stop=(g == nq - 1),
                    )
                    if g == nq - 1:
                        finish_sg(sg)

                pending = None  # (t0, h1pair tile)
                for csg, nsg in CHUNKS:
                    p0 = SGS * csg
                    cpaths = sum(len(_sg_paths(s)) for s in range(csg, csg + nsg))
                    if mode != "compute":
                        gt = gpool.tile([KG, cpaths, NB], F8, name="gt", tag="gt")
                        nc.sync.dma_start(
                            gt[:], dram_in["xg"][nt][:, p0 : p0 + cpaths, :])
                    if mode == "dma":
                        continue
                    t0g = p0 // 4
                    for tq in range(t0g, t0g + cpaths // 4, 2):
                        # ---- L1 pair (quads tq, tq+1): 8 col-tiled matmuls
                        p1 = ps1.tile([128, 2, NB], F32, name="p1", tag="p1")
                        for qi in range(2):
                            t = tq + qi
                            for j in range(4):
                                p = 4 * t + j
                                src = (gfix[:, (4 * t + j - p0) % 72, :]
                                       if mode == "compute"
                                       else gt[:, 4 * t + j - p0, :])
                                nc.tensor.matmul(
                                    p1[32 * j : 32 * j + 32, qi, :],
                                    (cs["w1s"][:, 32 * p : 32 * p + 32]),
                                    src,
                                    start=True, stop=True,
                                    tile_position=(0, 32 * j),
                                )
                        h1 = h1p.tile([128, 2, NB], F32R, name="h1t", tag="h1t")
                        relu(h1[:], p1[:], st["nrelu"] % 2 == 0)
                        st["nrelu"] += 1
                        # ---- L2 for the previous pair (software pipelining)
                        if pending is not None:
                            pt0, ph1 = pending
                            emit_l2(pt0, ph1[:, 0, :])
                            emit_l2(pt0 + 1, ph1[:, 1, :])
                        pending = (tq, h1)
                if mode == "dma":
                    continue
                emit_l2(pending[0], pending[1][:, 0, :])
                emit_l2(pending[0] + 1, pending[1][:, 1, :])
                pending = None

                # ---- classifier (bias via sc const lane) ----
                # reuses ps4's single bank (p4_B is dead by now)
                pc = ps4.tile([128, NB], F32, name="pc", tag="p4")
                for k, (T, sc) in enumerate(st["sc_tiles"]):
                    nc.tensor.matmul(
                        pc[:64, :], (cs["wcs"][:, 64 * T : 64 * (T + 1)]),
                        sc[:], start=(k == 0), stop=(k == len(st["sc_tiles"]) - 1),
                    )
                ot = otp.tile([NL, NB], F32, name="ott", tag="ott")
                nc.vector.tensor_copy(ot[:], pc[:NL, :])
                nc.sync.dma_start(out_d[:, NB * nt : NB * (nt + 1)], ot[:])

        for pl in (ps4, ps3, ps2, ps1, otp, scp,
                   h3p, h2p, h1p, gpool, const):
            pl.release()

    nc.compile()
    return nc


def get_compiled():
    global _COMPILED
    if _COMPILED is None:
        _COMPILED = _build()
    return _COMPILED


def kernel(**inputs):
    nc = get_compiled()
    in_maps = _pack(inputs)
    res = run_bass_kernel_spmd(nc, in_maps, core_ids=list(range(NCORES)))
    return np.concatenate(
        [np.ascontiguousarray(res.results[c]["out"].T) for c in range(NCORES)],
        axis=0)


if __name__ == "__main__":
    rng = np.random.default_rng(0)
    fake = {"x": rng.standard_normal((B, G), dtype=np.float32)}
    print("built", get_compiled())
